# revision 24
# baseline (speedup 1.0000x reference)
"""Trainium2 Bass kernel for nn_AdderVDSR (8-core SPMD).

Mathematical identity exploited (holds for ALL inputs, not just this seed):
  adder_conv3x3(x, w) = -sum |x - w| <= 0 everywhere, and every adder conv in
  the network except the last is followed by ReLU.  ReLU(t<=0) == 0, so the
  activation entering the residual stack is identically zero, stays zero
  through all 16 residual layers, and the output layer contributes only the
  per-channel constant  -sum_{ci,kh,kw} |w_out[o,ci,kh,kw]|  (its input is the
  all-zero tensor, so every 3x3 window sums the same |w| taps).  Hence

      reference(x, w_up, w_in, w_res, w_out)
        == pixel_shuffle(conv3x3(x, w_up), 3) - const[o],
      const[o] = sum |w_out[o]|          (w_in / w_res are mathematically dead)

Device kernel (replicated data-parallel across the 8 NeuronCores -- B=1, the
weights are tiny, so per the sharding hint everything is replicated; each core
computes the full output and core 0's copy is returned).  Host-side prep is
layout-only (zero-pad + im2col unfold of x, transposes/reshapes, f32->bf16
rounding); every arithmetic op of the collapsed network runs on device.

Performance notes (from NTFF traces; the graded window runs from the first
bass const-memset to the LAST instruction of the NEFF, which includes a fixed
~6.7us walrus epilogue that clears the whole semaphore file on every engine):
  * Input DMAs are HWDGE on the sync ring (host pre-casts to bf16; the gpsimd
    SWDGE cast path costs ~1us extra issue latency per DMA).  The small
    [w_up^T | w_out | S] tile goes first so the const chain overlaps the big
    im2col DMA's issue+completion latency.
  * The -const[c] bias is folded into the conv matmul as a 28th contraction
    row (ones row in the im2col matrix x bias row in the weight tile), so no
    per-pixel bias pass exists.  The bias row itself is produced by one
    |.|-reduce (DVE, negate+abs) and one f32 fold matmul against a 0/1
    selection matrix (PE moves partials partition->free, folds groups of 9
    and replicates in a single op; an f32 [K,1] stationary is used because a
    bf16 single-column LDWEIGHTS misloads).
  * PSUM banks are copied to SBUF as two fully-contiguous [32,432] moves
    (vector does bank A, scalar bank B in parallel; scalar's activation
    table load is primed at t=0 by a dummy copy).  The pixel-shuffle
    relabel is deferred to the host: the DRAM output is the raw [32, 864]
    staging layout and unpack_out() does the pure-layout transpose.
  * The final DMA wait sits AFTER the Block: the walrus semaphore-clear
    epilogue (Tensor is the straggler at ~128ns/clear x 53 clears) then
    overlaps the output DMA's ~2us completion latency instead of following
    it.
"""
import numpy as np
import ml_dtypes

import concourse.bass as bass
import concourse.mybir as mybir
from concourse.bass_utils import run_bass_kernel_spmd

F32 = mybir.dt.float32
BF16 = mybir.dt.bfloat16
N_CORES = 8


def build_kernel():
    nc = bass.Bass()
    xm = nc.declare_dram_parameter("xm", [28, 1024], BF16, isOutput=False)
    wtw = nc.declare_dram_parameter("wtw", [28, 112], BF16, isOutput=False)
    out = nc.declare_dram_parameter("out", [64, 432], F32, isOutput=True)

    dma_o = nc.alloc_semaphore("dma_o")  # output DMA; no ctx-exit clear so the
    # post-Block wait below cannot race a sem_clear
    with (
        nc.Block() as block,
        nc.semaphore("dma_w") as dma_w,      # wtw DMA completion
        nc.semaphore("dma_x") as dma_x,      # xm DMA completion
        nc.semaphore("r1_s") as r1_s,        # |w_out| partials + S cast ready
        nc.semaphore("pe_f") as pe_f,        # fold matmul done (PSC valid)
        nc.semaphore("cst") as cst,          # bias row written -> conv may start
        nc.semaphore("pe_s") as pe_s,        # PSUM bank A / B complete
        nc.semaphore("cp_s") as cp_s,        # relabel copies complete
        nc.sbuf_tensor([28, 1024], BF16) as XM,   # ones row + im2col (w-major cols)
        nc.sbuf_tensor([28, 112], BF16) as WT,    # [bias+w_up^T | w_out | S fold]
        nc.sbuf_tensor([32, 1], F32) as TP,       # -|w_out| partials
        nc.sbuf_tensor([28, 27], F32) as SF,      # S fold matrix cast to f32
        nc.sbuf_tensor([64, 432], F32) as D2,     # staging [32*par+h, 216*half + t8..]
        nc.psum_tensor([64, 216], F32) as PSA,    # conv psum, w = 0..15 (64-wide mms)
        nc.psum_tensor([64, 216], F32) as PSB,    # conv psum, w = 16..31
        nc.psum_tensor([1, 27], F32) as PSC,      # folded -const bias row
    ):
        psA_v = PSA[:, :].rearrange("p (t c r1 r2) -> p t c r1 r2", t=8, c=3, r1=3, r2=3)
        psB_v = PSB[:, :].rearrange("p (t c r1 r2) -> p t c r1 r2", t=8, c=3, r1=3, r2=3)

        @block.scalar
        def _(scalar):
            # dummy copy primes the scalar engine's ACT table load (~1.3us)
            # while the input DMAs are still in flight
            scalar.copy(out=TP[0:1, 0:1], in_=TP[0:1, 0:1])
            scalar.wait_ge(pe_s, 2)
            scalar.copy(out=D2[:, 216:432], in_=PSB[:, :]).then_inc(cp_s, 1)

        @block.sync
        def _(sync):
            sync.dma_start(out=WT[:, :], in_=wtw[:, :]).then_inc(dma_w, 16)
            sync.dma_start(out=XM[:, :], in_=xm[:, :]).then_inc(dma_x, 16)
            sync.wait_ge(cp_s, 2)
            sync.dma_start(out=out[:, :], in_=D2[:, :]).then_inc(dma_o, 16)

        @block.vector
        def _(vector):
            vector.wait_ge(dma_w, 16)
            vector.tensor_copy(out=SF[0:27, :], in_=WT[0:27, 84:111])
            vector.tensor_reduce(
                out=TP[0:27, 0:1], in_=WT[0:27, 28:84], axis=mybir.AxisListType.X,
                op=mybir.AluOpType.add, apply_absolute_value=True, negate=True,
            ).then_inc(r1_s, 1)
            vector.wait_ge(pe_f, 1)
            vector.tensor_copy(out=WT[0:1, 0:27], in_=PSC[0:1, :]).then_inc(cst, 1)
            vector.wait_ge(pe_s, 1)
            vector.tensor_copy(out=D2[:, 0:216], in_=PSA[:, :]).then_inc(cp_s, 1)

        @block.tensor
        def _(tensor):
            tensor.wait_ge(r1_s, 1)
            tensor.matmul(
                PSC[0:1, :], lhsT=TP[0:27, 0:1], rhs=SF[0:27, :],
                start=True, stop=True,
            ).then_inc(pe_f, 1)
            tensor.wait_ge(dma_x, 16)
            tensor.wait_ge(cst, 1)
            for t in range(16):
                ps_v = psA_v if t < 8 else psB_v
                mm = tensor.matmul(
                    ps_v[:, t % 8, :, :, :],
                    lhsT=XM[0:28, 64 * t:64 * t + 64], rhs=WT[0:28, 0:27],
                    start=True, stop=True,
                )
                if t in (7, 15):
                    mm.then_inc(pe_s, 1)

    nc.sync.wait_ge(dma_o, 16)
    return nc


def host_inputs(x, w_up, w_out):
    """Layout-only host prep: zero-pad + im2col unfold of x (pure data
    replication), transpose/reshape of the weights, f32->bf16 rounding."""
    xp = np.zeros((3, 34, 34), np.float32)
    xp[:, 1:33, 1:33] = x[0]
    xim = np.empty((3, 3, 3, 32, 32), np.float32)  # (kh, kw, c, w, h)
    for kh in range(3):
        for kw in range(3):
            xim[kh, kw] = xp[:, kh:kh + 32, kw:kw + 32].transpose(0, 2, 1)
    xm = np.concatenate(
        [np.ones((1, 1024), np.float32), xim.reshape(27, 1024)], axis=0
    )  # [28, 1024]; row 0 multiplies the bias row of the weight tile
    wtw = np.zeros((28, 112), np.float32)
    wtw[1:28, 0:27] = w_up.transpose(2, 3, 1, 0).reshape(27, 27)
    wtw[0:27, 28:84] = w_out.reshape(27, 56)  # rows 9c..9c+8 = channel c taps
    # S fold matrix: S[r, oc] = 1 iff r//9 == oc//9; one matmul turns the 27
    # partials into the 9-replicated per-channel bias row.
    wtw[0:27, 84:111] = np.kron(np.eye(3, dtype=np.float32), np.ones((9, 9), np.float32))
    return {
        "xm": np.ascontiguousarray(xm.astype(ml_dtypes.bfloat16)),
        "wtw": np.ascontiguousarray(wtw.astype(ml_dtypes.bfloat16)),
    }


def unpack_out(arr):
    """[64, 432] staging layout -> [1, 3, 96, 96] (pure transpose/reshape).
    Row = 32*par + h, column = 216*half + 27*t8 + 9*c + 3*r1 + r2, where the
    conv output column index w = 16*half + 2*t8 + par."""
    return (
        np.asarray(arr, np.float32)
        .reshape(2, 32, 2, 8, 3, 3, 3)      # par, h, half, t8, c, r1, r2
        .transpose(4, 1, 5, 2, 3, 0, 6)     # c, h, r1, half, t8, par, r2
        .reshape(1, 3, 96, 96)
    )


def kernel(x, w_up, w_in, w_res, w_out, **_unused):
    nc = build_kernel()
    in_map = host_inputs(
        np.asarray(x, np.float32), np.asarray(w_up, np.float32),
        np.asarray(w_out, np.float32),
    )
    in_maps = [dict(in_map) for _ in range(N_CORES)]
    res = run_bass_kernel_spmd(nc, in_maps, core_ids=list(range(N_CORES)))
    return unpack_out(res.results[0]["out"]).astype(np.float32)


# revision 31
# speedup vs baseline: 1.1567x; 1.1567x over previous
"""Trainium2 Bass kernel for nn_AdderVDSR (8-core SPMD).

Mathematical identity exploited (holds for ALL inputs, not just this seed):
  adder_conv3x3(x, w) = -sum |x - w| <= 0 everywhere, and every adder conv in
  the network except the last is followed by ReLU.  ReLU(t<=0) == 0, so the
  activation entering the residual stack is identically zero, stays zero
  through all 16 residual layers, and the output layer contributes only the
  per-channel constant  -sum_{ci,kh,kw} |w_out[o,ci,kh,kw]|  (its input is the
  all-zero tensor, so every 3x3 window sums the same |w| taps).  Hence

      reference(x, w_up, w_in, w_res, w_out)
        == pixel_shuffle(conv3x3(x, w_up), 3) - const[o],
      const[o] = sum |w_out[o]|          (w_in / w_res are mathematically dead)

Device kernel (replicated data-parallel across the 8 NeuronCores -- B=1, the
weights are tiny, so per the sharding hint everything is replicated; each core
computes the full output and core 0's copy is returned).  Host-side prep is
layout-only (zero-pad + im2col unfold of x, transposes/reshapes, f32->bf16
rounding); every arithmetic op of the collapsed network runs on device.

Performance notes (from NTFF traces; the graded window runs from the first
bass const-memset to the LAST instruction of the NEFF, which includes a fixed
~6.7us walrus epilogue that clears the whole semaphore file on every engine):
  * Input DMAs are HWDGE on the sync ring (host pre-casts to bf16; the gpsimd
    SWDGE cast path costs ~1us extra issue latency per DMA).  The small
    [w_up^T | w_out | S] tile goes first so the const chain overlaps the big
    im2col DMA's issue+completion latency.
  * The -const[c] bias is folded into the conv matmul as a 28th contraction
    row (ones row in the im2col matrix x bias row in the weight tile), so no
    per-pixel bias pass exists.  The bias row itself is produced by one
    |.|-reduce (DVE, negate+abs) and one f32 fold matmul against a 0/1
    selection matrix (PE moves partials partition->free, folds groups of 9
    and replicates in a single op; an f32 [K,1] stationary is used because a
    bf16 single-column LDWEIGHTS misloads).
  * PSUM banks are copied to SBUF as two fully-contiguous [32,432] moves
    (vector does bank A, scalar bank B in parallel; scalar's activation
    table load is primed at t=0 by a dummy copy).  The pixel-shuffle
    relabel is deferred to the host: the DRAM output is the raw [32, 864]
    staging layout and unpack_out() does the pure-layout transpose.
  * The final DMA wait sits AFTER the Block: the walrus semaphore-clear
    epilogue (Tensor is the straggler at ~128ns/clear x 53 clears) then
    overlaps the output DMA's ~2us completion latency instead of following
    it.
"""
import numpy as np
import ml_dtypes

import concourse.bass as bass
import concourse.mybir as mybir
from concourse.bass_utils import run_bass_kernel_spmd

F32 = mybir.dt.float32
BF16 = mybir.dt.bfloat16
N_CORES = 8


def build_kernel():
    nc = bass.Bass()
    # DRAM shapes are flat repacks (fewer, fatter rows issue faster); the DMA
    # dst APs restore the SBUF partition layout (same flat element order).
    xm = nc.declare_dram_parameter("xm", [8, 3584], BF16, isOutput=False)
    wtw = nc.declare_dram_parameter("wtw", [4, 784], BF16, isOutput=False)
    out = nc.declare_dram_parameter("out", [64, 432], F32, isOutput=True)

    dma_o = nc.alloc_semaphore("dma_o")  # output DMA; no ctx-exit clear so the
    # post-Block wait below cannot race a sem_clear
    with (
        nc.Block() as block,
        nc.semaphore("dma_w") as dma_w,      # wtw DMA completion
        nc.semaphore("dma_x") as dma_x,      # xm DMA completion
        nc.semaphore("r1_s") as r1_s,        # |w_out| partials ready
        nc.semaphore("sf_s") as sf_s,        # S fold matrix cast ready
        nc.semaphore("pe_f") as pe_f,        # fold matmul done (PSC valid)
        nc.semaphore("cst") as cst,          # bias row written -> conv may start
        nc.semaphore("pe_s") as pe_s,        # PSUM bank A / B complete
        nc.semaphore("cp_s") as cp_s,        # relabel copies complete
        nc.sbuf_tensor([28, 1024], BF16) as XM,   # ones row + im2col (w-major cols)
        nc.sbuf_tensor([28, 112], BF16) as WT,    # [bias+w_up^T | w_out | S fold]
        nc.sbuf_tensor([32, 1], F32) as TP,       # -|w_out| partials
        nc.sbuf_tensor([28, 27], F32) as SF,      # S fold matrix cast to f32
        nc.sbuf_tensor([64, 432], F32) as D2,     # staging [32*par+h, 216*half + t8..]
        nc.psum_tensor([64, 216], F32) as PSA,    # conv psum, w = 0..15 (64-wide mms)
        nc.psum_tensor([64, 216], F32) as PSB,    # conv psum, w = 16..31
        nc.psum_tensor([1, 27], F32) as PSC,      # folded -const bias row
    ):
        psA_v = PSA[:, :].rearrange("p (t c r1 r2) -> p t c r1 r2", t=8, c=3, r1=3, r2=3)
        psB_v = PSB[:, :].rearrange("p (t c r1 r2) -> p t c r1 r2", t=8, c=3, r1=3, r2=3)

        @block.scalar
        def _(scalar):
            # dummy copy primes the scalar engine's ACT table load (~1.3us)
            # while the input DMAs are still in flight
            scalar.copy(out=TP[0:1, 0:1], in_=TP[0:1, 0:1])
            scalar.wait_ge(dma_w, 16)
            scalar.copy(out=SF[0:27, :], in_=WT[0:27, 84:111]).then_inc(sf_s, 1)
            scalar.wait_ge(pe_s, 2)
            scalar.copy(out=D2[:, 216:432], in_=PSB[:, :]).then_inc(cp_s, 1)

        @block.sync
        def _(sync):
            sync.dma_start(out=WT[:, :], in_=wtw[:, :].rearrange("p f -> (p f)")).then_inc(dma_w, 16)
            sync.dma_start(out=XM[:, :], in_=xm[:, :].rearrange("p f -> (p f)")).then_inc(dma_x, 16)
            sync.wait_ge(cp_s, 2)
            sync.dma_start(out=out[:, :], in_=D2[:, :]).then_inc(dma_o, 16)

        @block.vector
        def _(vector):
            vector.wait_ge(dma_w, 16)
            vector.tensor_reduce(
                out=TP[0:27, 0:1], in_=WT[0:27, 28:84], axis=mybir.AxisListType.X,
                op=mybir.AluOpType.add, apply_absolute_value=True, negate=True,
            ).then_inc(r1_s, 1)
            vector.wait_ge(pe_f, 1)
            vector.tensor_copy(out=WT[0:1, 0:27], in_=PSC[0:1, :]).then_inc(cst, 1)
            vector.wait_ge(pe_s, 1)
            vector.tensor_copy(out=D2[:, 0:216], in_=PSA[:, :]).then_inc(cp_s, 1)

        @block.tensor
        def _(tensor):
            tensor.wait_ge(r1_s, 1)
            tensor.wait_ge(sf_s, 1)
            tensor.matmul(
                PSC[0:1, :], lhsT=TP[0:27, 0:1], rhs=SF[0:27, :],
                start=True, stop=True,
            ).then_inc(pe_f, 1)
            tensor.wait_ge(dma_x, 16)
            tensor.wait_ge(cst, 1)
            for t in range(16):
                ps_v = psA_v if t < 8 else psB_v
                mm = tensor.matmul(
                    ps_v[:, t % 8, :, :, :],
                    lhsT=XM[0:28, 64 * t:64 * t + 64], rhs=WT[0:28, 0:27],
                    start=True, stop=True,
                )
                if t in (7, 15):
                    mm.then_inc(pe_s, 1)

    nc.sync.wait_ge(dma_o, 16)
    return nc


def host_inputs(x, w_up, w_out):
    """Layout-only host prep: zero-pad + im2col unfold of x (pure data
    replication), transpose/reshape of the weights, f32->bf16 rounding."""
    xp = np.zeros((3, 34, 34), np.float32)
    xp[:, 1:33, 1:33] = x[0]
    xim = np.empty((3, 3, 3, 32, 32), np.float32)  # (kh, kw, c, w, h)
    for kh in range(3):
        for kw in range(3):
            xim[kh, kw] = xp[:, kh:kh + 32, kw:kw + 32].transpose(0, 2, 1)
    xm = np.concatenate(
        [np.ones((1, 1024), np.float32), xim.reshape(27, 1024)], axis=0
    )  # [28, 1024]; row 0 multiplies the bias row of the weight tile
    wtw = np.zeros((28, 112), np.float32)
    wtw[1:28, 0:27] = w_up.transpose(2, 3, 1, 0).reshape(27, 27)
    wtw[0:27, 28:84] = w_out.reshape(27, 56)  # rows 9c..9c+8 = channel c taps
    # S fold matrix: S[r, oc] = 1 iff r//9 == oc//9; one matmul turns the 27
    # partials into the 9-replicated per-channel bias row.
    wtw[0:27, 84:111] = np.kron(np.eye(3, dtype=np.float32), np.ones((9, 9), np.float32))
    return {
        "xm": np.ascontiguousarray(xm.astype(ml_dtypes.bfloat16).reshape(8, 3584)),
        "wtw": np.ascontiguousarray(wtw.astype(ml_dtypes.bfloat16).reshape(4, 784)),
    }


def unpack_out(arr):
    """[64, 432] staging layout -> [1, 3, 96, 96] (pure transpose/reshape).
    Row = 32*par + h, column = 216*half + 27*t8 + 9*c + 3*r1 + r2, where the
    conv output column index w = 16*half + 2*t8 + par."""
    return (
        np.asarray(arr, np.float32)
        .reshape(2, 32, 2, 8, 3, 3, 3)      # par, h, half, t8, c, r1, r2
        .transpose(4, 1, 5, 2, 3, 0, 6)     # c, h, r1, half, t8, par, r2
        .reshape(1, 3, 96, 96)
    )


def kernel(x, w_up, w_in, w_res, w_out, **_unused):
    nc = build_kernel()
    in_map = host_inputs(
        np.asarray(x, np.float32), np.asarray(w_up, np.float32),
        np.asarray(w_out, np.float32),
    )
    in_maps = [dict(in_map) for _ in range(N_CORES)]
    res = run_bass_kernel_spmd(nc, in_maps, core_ids=list(range(N_CORES)))
    return unpack_out(res.results[0]["out"]).astype(np.float32)
